# revision 1
# baseline (speedup 1.0000x reference)
"""Trainium2 Bass kernel for bidirectional Chamfer distance (B=8, N=M=8192).

Sharding: data-parallel over batch -- one NeuronCore per batch element; the
host combines the 8 cores' per-point minima (all-reduce of the scalar means
is O(N) host work).

Per core, both directions of the chamfer min run as two matmul orientations
(weights=targets / weights=preds) of an augmented K=24 matmul that emits
finished 128x512 squared-distance tiles straight into PSUM:

    dist(n, m) = p_sq[n] + t_sq[m] - 2 <p_n, t_m>

Numerics: every augmented row is split into three bf16 parts (hi/mid/lo), so
each fp32 input is represented exactly to ~2^-25 and all bf16 products are
exact in the PE's fp32 accumulate -> fp32-level accuracy at bf16 streaming
speed (1 cycle/row).  K=24 <= 32 lets four matmuls run concurrently in the
PE's four 32-row groups (tile_position=(32i,0)), one PSUM bank each (~4x PE
throughput).

Reduction: the Vector engine is the only min-capable engine, and its
tensor_tensor_scan(op0=min, op1=min) folds one PSUM tile + one SBUF tile
(staged by the Scalar engine from another PSUM bank) per instruction -- the
best PSUM-drain rate available -- with a [128,1] carry chaining the running
min across the stream dimension.  Host applies max(.,0) + means.
"""

import ml_dtypes
import numpy as np

import concourse.bass as bass
import concourse.mybir as mybir
import concourse.tile as tile
from concourse import bacc
from concourse.bass_utils import run_bass_kernel_spmd

try:  # persistent jit/NEFF cache: makes repeat invocations fast
    import jax

    jax.config.update("jax_compilation_cache_dir", "/tmp/.jax_bass_cache")
    jax.config.update("jax_persistent_cache_min_compile_time_secs", 1.0)
except Exception:
    pass

F32 = mybir.dt.float32
F16 = mybir.dt.float16
BF16 = mybir.dt.bfloat16
MIN = mybir.AluOpType.min
BIG = 3.0e38

B, N, M = 8, 8192, 8192
KROWS = 24
CHUNK = 512
GROUP = 2  # 512-col chunks per scan operand (scan free-dim = GROUP*CHUNK)


def _build_nc(N=8192, M=8192, group=2, chunk=512, repeat=1, scan_bufs=3, cp_bufs=3, hybrid=True):
    """Inputs (per core), all [128, n] bf16 with the 24 aug rows replicated at
    partition offsets 0/32/64/96:
      wa: aug-weights(target) [128, M]   (orientation A: out[m_part, n_free])
      sa: aug-stream(pred)    [128, N]
      wb: aug-weights(pred)   [128, N]   (orientation B: out[n_part, m_free])
      sb: aug-stream(target)  [128, M]
    Output: mins [128, M/128 + N/128] fp32.
    """
    assert N % (2 * group * chunk) == 0 and M % (2 * group * chunk) == 0
    nta = M // 128
    ntb = N // 128
    fd = group * chunk
    assert group == 2, "row-group packing assumes 4 chunks (2 groups) per iter"

    nc = bacc.Bacc("TRN2", target_bir_lowering=False, debug=False)
    wa = nc.dram_tensor("wa", [128, M], BF16, kind="ExternalInput").ap()
    sa = nc.dram_tensor("sa", [128, N], BF16, kind="ExternalInput").ap()
    wb = nc.dram_tensor("wb", [128, N], BF16, kind="ExternalInput").ap()
    sb = nc.dram_tensor("sb", [128, M], BF16, kind="ExternalInput").ap()
    out = nc.dram_tensor("mins", [128, nta + ntb], F32, kind="ExternalOutput").ap()

    with tile.TileContext(nc) as tc:
        with (
            tc.tile_pool(name="const", bufs=1) as const_pool,
            tc.tile_pool(name="psum", bufs=(2 if hybrid else 4), space="PSUM") as psum_pool,
            tc.tile_pool(name="psum2", bufs=2, space="PSUM") as psum2_pool,
            tc.tile_pool(name="f16", bufs=6) as f16_pool,
            tc.tile_pool(name="cp", bufs=cp_bufs) as copy_pool,
            tc.tile_pool(name="scan", bufs=scan_bufs) as scan_pool,
            tc.tile_pool(name="res", bufs=1) as res_pool,
        ):
            sb_t = {}
            for name, dram in (("wa", wa), ("sa", sa), ("wb", wb), ("sb", sb)):
                t = const_pool.tile([128, dram.shape[1]], BF16, tag=name)
                nc.sync.dma_start(t[:], dram[:])
                sb_t[name] = t

            res = res_pool.tile([128, nta + ntb], F32)

            for _rep in range(repeat):
              for wname, sname, ntiles, col0 in (
                ("wa", "sa", nta, 0),
                ("wb", "sb", ntb, nta),
              ):
                w = sb_t[wname]
                s = sb_t[sname]
                n_stream = s.shape[1]
                nchunks = n_stream // chunk
                niter = nchunks // (2 * group)
                for t in range(ntiles):
                    carry = None
                    scan_iters = niter // 4 if hybrid else niter
                    for g in range(scan_iters):
                        ps0 = psum_pool.tile([128, fd], F32, tag="ps")
                        ps1 = psum_pool.tile([128, fd], F32, tag="ps")
                        base = g * 2 * group
                        # 4 chunks -> 4 concurrent row-group matmuls,
                        # one PSUM bank each
                        for i, (pst, j) in enumerate(
                            ((ps0, 0), (ps0, 1), (ps1, 0), (ps1, 1))
                        ):
                            c = base + i
                            rp = 32 * i
                            nc.tensor.matmul(
                                pst[:, j * chunk : (j + 1) * chunk],
                                lhsT=w[rp : rp + KROWS, t * 128 : (t + 1) * 128],
                                rhs=s[rp : rp + KROWS, c * chunk : (c + 1) * chunk],
                                start=True,
                                stop=True,
                                tile_position=(rp, 0),
                            )
                        cp = copy_pool.tile([128, fd], F32, tag="cp")
                        nc.scalar.copy(cp[:], ps1[:])
                        so = scan_pool.tile([128, fd], F32, tag="so")
                        init = BIG if carry is None else carry
                        nc.vector.tensor_tensor_scan(
                            so[:], ps0[:], cp[:], init, op0=MIN, op1=MIN
                        )
                        carry = so[:, fd - 1 : fd]
                    if hybrid:
                        # remaining chunks via fp16 fast path: ACT casts each
                        # 4-bank PSUM tile to fp16; DVE folds with 2x-mode TT
                        leaves = []
                        n_leaves = (nchunks - scan_iters * 2 * group) // 2
                        for h in range(n_leaves):
                            psb = psum2_pool.tile([128, 2 * chunk], F32, tag="psb")
                            base = scan_iters * 2 * group + h * 2
                            for i in range(2):
                                c = base + i
                                rp = 32 * ((h % 2) * 2 + i)
                                nc.tensor.matmul(
                                    psb[:, i * chunk : (i + 1) * chunk],
                                    lhsT=w[rp : rp + KROWS, t * 128 : (t + 1) * 128],
                                    rhs=s[rp : rp + KROWS, c * chunk : (c + 1) * chunk],
                                    start=True,
                                    stop=True,
                                    tile_position=(32 * ((h % 2) * 2 + i), 0),
                                )
                            lf = f16_pool.tile([128, 2 * chunk], F16, tag="leaf")
                            nc.scalar.copy(lf[:], psb[:])
                            leaves.append(lf)
                        while len(leaves) > 1:
                            nxt = []
                            for a, b in zip(leaves[::2], leaves[1::2]):
                                m = f16_pool.tile([128, 2 * chunk], F16, tag="m16")
                                nc.vector.tensor_tensor(m[:], a[:], b[:], op=MIN)
                                nxt.append(m)
                            if len(leaves) % 2:
                                nxt.append(leaves[-1])
                            leaves = nxt
                        m16 = leaves[0]
                        f1 = f16_pool.tile([128, chunk], F16, tag="f1")
                        nc.vector.tensor_tensor(
                            f1[:], m16[:, :chunk], m16[:, chunk :], op=MIN
                        )
                        f2 = f16_pool.tile([128, chunk // 2], F16, tag="f2")
                        nc.vector.tensor_tensor(
                            f2[:], f1[:, : chunk // 2], f1[:, chunk // 2 :], op=MIN
                        )
                        fmin = f16_pool.tile([128, 1], F32, tag="fmin")
                        nc.vector.tensor_reduce(fmin[:], f2[:], axis=mybir.AxisListType.X, op=MIN)
                        nc.vector.tensor_tensor(res[:, col0 + t : col0 + t + 1], fmin[:], carry, op=MIN)
                    else:
                        nc.scalar.copy(res[:, col0 + t : col0 + t + 1], carry)

            nc.sync.dma_start(out[:], res[:])

    nc.compile()
    return nc


def _split3(x):
    """fp32 -> (hi, mid, lo) bf16 parts with hi+mid+lo == x to ~2^-25 rel."""
    x = np.asarray(x, np.float32)
    h = x.astype(ml_dtypes.bfloat16)
    r = x - h.astype(np.float32)
    m = r.astype(ml_dtypes.bfloat16)
    l = (r - m.astype(np.float32)).astype(ml_dtypes.bfloat16)
    return h, m, l


def _aug24(w_pts, s_pts, w_sq, s_sq):
    """K=24 bf16 weight/stream matrices for one orientation (w side gets -2)."""
    Mw = w_pts.shape[0]
    Ns = s_pts.shape[0]
    W = np.zeros((KROWS, Mw), ml_dtypes.bfloat16)
    S = np.zeros((KROWS, Ns), ml_dtypes.bfloat16)
    one_w = np.ones(Mw, ml_dtypes.bfloat16)
    one_s = np.ones(Ns, ml_dtypes.bfloat16)

    W[0], W[1], W[2] = _split3(w_sq)
    S[0], S[1], S[2] = one_s, one_s, one_s
    W[3], W[4], W[5] = one_w, one_w, one_w
    S[3], S[4], S[5] = _split3(s_sq)

    for c in range(3):
        vh, vm, vl = _split3((-2.0 * w_pts[:, c]).astype(np.float32))
        ph, pm, pl = _split3(s_pts[:, c])
        r = 6 + 6 * c
        W[r + 0], S[r + 0] = vh, ph
        W[r + 1], S[r + 1] = vh, pm
        W[r + 2], S[r + 2] = vm, ph
        W[r + 3], S[r + 3] = vh, pl
        W[r + 4], S[r + 4] = vl, ph
        W[r + 5], S[r + 5] = vm, pm
    return W, S


def _replicate4(A):
    """[24, n] -> [128, n] with copies at partition offsets 0/32/64/96."""
    out = np.zeros((128, A.shape[1]), ml_dtypes.bfloat16)
    for i in range(4):
        out[32 * i : 32 * i + KROWS] = A
    return out


def _augment(pred_b, target_b):
    """Host-side O(N) prep for one batch -> four [128, n] bf16 arrays."""
    p = np.asarray(pred_b, np.float32)
    t = np.asarray(target_b, np.float32)
    p_sq = (p.astype(np.float64) ** 2).sum(axis=1).astype(np.float32)
    t_sq = (t.astype(np.float64) ** 2).sum(axis=1).astype(np.float32)
    WA, SA = _aug24(t, p, t_sq, p_sq)  # orientation A: weights = targets
    WB, SB = _aug24(p, t, p_sq, t_sq)  # orientation B: weights = preds
    return {
        "wa": _replicate4(WA),
        "sa": _replicate4(SA),
        "wb": _replicate4(WB),
        "sb": _replicate4(SB),
    }


_NC_CACHE = {}


def _get_nc():
    if "nc" not in _NC_CACHE:
        _NC_CACHE["nc"] = _build_nc()
    return _NC_CACHE["nc"]


def kernel(pred: np.ndarray, target: np.ndarray) -> np.ndarray:
    pred = np.asarray(pred, np.float32)
    target = np.asarray(target, np.float32)
    assert pred.shape == (B, N, 3) and target.shape == (B, M, 3), (
        pred.shape,
        target.shape,
    )

    nc = _get_nc()
    in_maps = [_augment(pred[b], target[b]) for b in range(B)]
    results = run_bass_kernel_spmd(nc, in_maps, list(range(B))).results

    nta = M // 128
    t2p = []  # per-target minima (min over preds)
    p2t = []  # per-pred minima (min over targets)
    for b in range(B):
        mins = results[b]["mins"]
        t2p.append(np.maximum(mins[:, :nta], 0.0).reshape(-1))
        p2t.append(np.maximum(mins[:, nta:], 0.0).reshape(-1))
    cd = np.mean(np.concatenate(p2t), dtype=np.float64) + np.mean(
        np.concatenate(t2p), dtype=np.float64
    )
    return np.array(cd, dtype=np.float32)



# revision 11
# speedup vs baseline: 2.7284x; 2.7284x over previous
"""Trainium2 Bass kernel for bidirectional Chamfer distance (B=8, N=M=8192).

Sharding: data-parallel over batch -- one NeuronCore per batch element.

Algorithm (exact, certificate-pruned):
  Points are sorted by radius |x| on the host. After sorting, each 128-pred
  tile i only computes distances to a static rank-centered window of W=1024
  targets (A[i] = clamp(128*i - 448, 0, M - W)): identically-distributed
  sorted samples rank-align, so nearest neighbors are rank-near for all but a
  handful of void/outlier points. Those are detected exactly on the host with
  a grid-bucket witness u_P (distance to some spatially-near other-set point
  bounds the nn distance, which bounds the nn's |key| offset, which bounds its
  rank window); points whose certificate fails ("hard", ~20 of 8192) are
  gathered into one extra 128-wide tile per direction that computes its full
  8192-point rows. Host min-combines, so the result is exact up to fp16
  rounding of individual distances (rel err ~3e-6 on the mean).

Per regular tile the device does: 2 quadrant-packed K=24 augmented matmuls
(exact bf16-triple-split, as before) -> PSUM [128,1024] fp32; ScalarE casts
to fp16 SBUF; VectorE folds the tile both ways: a fused tensor_tensor_reduce
gives the per-pred row min in one op, and an in-place tensor_tensor(min)
updates a [128, 8192] fp16 column accumulator (first-touch regions use
tensor_copy). Fully-covered 128-col blocks of the accumulator are finalized
inline via PE transpose + VectorE free-dim reduce. Hard tiles reuse the same
pipeline with full windows and no column work. Host applies max(.,0) + means.
"""

import numpy as np
import ml_dtypes

import concourse.bass as bass
import concourse.mybir as mybir
import concourse.tile as tile
from concourse import bacc
from concourse.bass_utils import run_bass_kernel_spmd

try:  # persistent jit/NEFF cache: makes repeat invocations fast
    import jax

    jax.config.update("jax_compilation_cache_dir", "/tmp/.jax_bass_cache")
    jax.config.update("jax_persistent_cache_min_compile_time_secs", 1.0)
except Exception:
    pass

F32 = mybir.dt.float32
F16 = mybir.dt.float16
BF16 = mybir.dt.bfloat16
MIN = mybir.AluOpType.min
BIG = 3.0e38

B, N, M = 8, 8192, 8192
KROWS = 24
W = 1024  # regular window width
NT = N // 128  # 64 pred tiles
NH = 8  # hard sub-tiles per direction (8 x 1024 = full row)
HARD_CAP = 128  # hard points per direction (one gathered tile)

# static rank-centered windows, 64-granular
A_WIN = [min(max(128 * i - 448, 0), M - W) for i in range(NT)]


def _block_finalizers():
    """last regular tile whose window fully covers acc block b -> finalize."""
    fin = {i: [] for i in range(NT)}
    for b in range(M // 128):
        last = -1
        for i in range(NT):
            if A_WIN[i] <= 128 * b and A_WIN[i] + W >= 128 * (b + 1):
                last = i
        assert last >= 0, f"acc block {b} uncovered"
        fin[last].append(b)
    return fin


_FINALIZE = _block_finalizers()


def build_nc(repeat=1, use_ttr=False, use_transpose=True, use_colacc=True, **_ignored):
    """Inputs (per core), bf16 [128, n] with 24 aug rows replicated at
    partition offsets 0/32/64/96 (except ident):
      wa:  aug-weights(pred)         [128, N]
      sa:  aug-stream(target)        [128, M]
      wah: aug-weights(hard preds)   [128, 128]
      wbh: aug-weights(hard targets) [128, 128]
      sb:  aug-stream(pred)          [128, N]
      ident: fp16 identity           [128, 128]
    Output: mins [128, 144] fp32:
      [:, 0:64]    row min per regular tile (per sorted pred)
      [:, 64:128]  col min per 128-target block (per sorted target)
      [:, 128:136] hard-pred partial row mins (8 sub-tiles, host folds)
      [:, 136:144] hard-target partial row mins
    """
    nc = bacc.Bacc("TRN2", target_bir_lowering=False, debug=False)
    wa = nc.dram_tensor("wa", [128, N], BF16, kind="ExternalInput").ap()
    sa = nc.dram_tensor("sa", [128, M], BF16, kind="ExternalInput").ap()
    wah = nc.dram_tensor("wah", [128, 128], BF16, kind="ExternalInput").ap()
    wbh = nc.dram_tensor("wbh", [128, 128], BF16, kind="ExternalInput").ap()
    sb = nc.dram_tensor("sb", [128, N], BF16, kind="ExternalInput").ap()
    ident = nc.dram_tensor("ident", [128, 128], F16, kind="ExternalInput").ap()
    out = nc.dram_tensor("mins", [128, 144], F32, kind="ExternalOutput").ap()

    with tile.TileContext(nc) as tc:
        with (
            tc.tile_pool(name="const", bufs=1) as const_pool,
            tc.tile_pool(name="acc", bufs=1) as acc_pool,
            tc.tile_pool(name="res", bufs=1) as res_pool,
            tc.tile_pool(name="psum", bufs=3, space="PSUM") as psum_pool,
            tc.tile_pool(name="psT", bufs=2, space="PSUM") as psT_pool,
            tc.tile_pool(name="sf", bufs=4) as sf_pool,
            tc.tile_pool(name="rt", bufs=2) as rt_pool,
        ):
            sb_t = {}
            for name, dram in (
                ("wa", wa),
                ("sa", sa),
                ("wah", wah),
                ("wbh", wbh),
                ("sb", sb),
                ("ident", ident),
            ):
                t = const_pool.tile(
                    [128, dram.shape[1]],
                    F16 if name == "ident" else BF16,
                    tag=name,
                )
                nc.sync.dma_start(t[:], dram[:])
                sb_t[name] = t

            acc = acc_pool.tile([128, M], F16)
            res = res_pool.tile([128, 144], F32)

            def tile_body(qbase, w_t, wcol, s_t, win, res_col, col_update, prev_end):
                """One [128 pts] x [win, win+W) window: matmuls -> cast ->
                row min into res[:, res_col]; optional col acc update."""
                ps = psum_pool.tile([128, W], F32, tag="ps")
                for j in range(2):
                    q = (qbase + j) % 4
                    rp = 32 * q
                    nc.tensor.matmul(
                        ps[:, j * 512 : (j + 1) * 512],
                        lhsT=w_t[rp : rp + KROWS, wcol : wcol + 128],
                        rhs=s_t[rp : rp + KROWS, win + j * 512 : win + (j + 1) * 512],
                        start=True,
                        stop=True,
                        tile_position=(rp, 0),
                    )
                sf = sf_pool.tile([128, W], F16, tag="sf")
                nc.scalar.copy(sf[:], ps[:])
                if col_update and use_colacc:
                    old_w = min(prev_end, win + W) - win
                    if old_w > 0:
                        nc.vector.tensor_tensor(
                            acc[:, win : win + old_w],
                            sf[:, :old_w],
                            acc[:, win : win + old_w],
                            op=MIN,
                        )
                    if old_w < W:
                        nc.vector.tensor_copy(
                            acc[:, win + old_w : win + W], sf[:, old_w:W]
                        )
                rt = rt_pool.tile([128, 512], F16, tag="rt")
                if use_ttr:
                    nc.vector.tensor_tensor_reduce(
                        rt[:],
                        sf[:, :512],
                        sf[:, 512:],
                        1.0,
                        6.0e4,
                        op0=MIN,
                        op1=MIN,
                        accum_out=res[:, res_col : res_col + 1],
                    )
                else:
                    nc.vector.tensor_tensor(rt[:], sf[:, :512], sf[:, 512:], op=MIN)
                    nc.vector.tensor_tensor(
                        rt[:, :256], rt[:, :256], rt[:, 256:], op=MIN
                    )
                    nc.vector.tensor_reduce(
                        res[:, res_col : res_col + 1],
                        rt[:, :256],
                        axis=mybir.AxisListType.X,
                        op=MIN,
                    )

            for _rep in range(repeat):
                prev_end = 0
                for i in range(NT):
                    tile_body(
                        qbase=2 * i,
                        w_t=sb_t["wa"],
                        wcol=128 * i,
                        s_t=sb_t["sa"],
                        win=A_WIN[i],
                        res_col=i,
                        col_update=True,
                        prev_end=prev_end,
                    )
                    prev_end = A_WIN[i] + W
                    if not use_transpose:
                        continue
                    for b in _FINALIZE[i]:
                        pt = psT_pool.tile([128, 1024], F16, tag="pt")
                        nc.tensor.transpose(
                            pt[:, 0:128],
                            acc[:, 128 * b : 128 * (b + 1)],
                            sb_t["ident"][:],
                        )
                        nc.vector.tensor_reduce(
                            res[:, 64 + b : 65 + b],
                            pt[:, 0:128],
                            axis=mybir.AxisListType.X,
                            op=MIN,
                        )
                for k in range(NH):
                    tile_body(
                        qbase=2 * k,
                        w_t=sb_t["wah"],
                        wcol=0,
                        s_t=sb_t["sa"],
                        win=W * k,
                        res_col=128 + k,
                        col_update=False,
                        prev_end=0,
                    )
                for k in range(NH):
                    tile_body(
                        qbase=2 * k + 1,
                        w_t=sb_t["wbh"],
                        wcol=0,
                        s_t=sb_t["sb"],
                        win=W * k,
                        res_col=136 + k,
                        col_update=False,
                        prev_end=0,
                    )

            nc.sync.dma_start(out[:], res[:])

    nc.compile()
    return nc


# ---------------------------------------------------------------------------
# host-side prep
# ---------------------------------------------------------------------------


def _split3(x):
    """fp32 -> (hi, mid, lo) bf16 parts with hi+mid+lo == x to ~2^-25 rel."""
    x = np.asarray(x, np.float32)
    h = x.astype(ml_dtypes.bfloat16)
    r = x - h.astype(np.float32)
    m = r.astype(ml_dtypes.bfloat16)
    l = (r - m.astype(np.float32)).astype(ml_dtypes.bfloat16)
    return h, m, l


def _aug24(w_pts, s_pts, w_sq, s_sq):
    """K=24 bf16 weight/stream matrices for one orientation (w side gets -2)."""
    Mw = w_pts.shape[0]
    Ns = s_pts.shape[0]
    Wm = np.zeros((KROWS, Mw), ml_dtypes.bfloat16)
    S = np.zeros((KROWS, Ns), ml_dtypes.bfloat16)
    one_w = np.ones(Mw, ml_dtypes.bfloat16)
    one_s = np.ones(Ns, ml_dtypes.bfloat16)

    Wm[0], Wm[1], Wm[2] = _split3(w_sq)
    S[0], S[1], S[2] = one_s, one_s, one_s
    Wm[3], Wm[4], Wm[5] = one_w, one_w, one_w
    S[3], S[4], S[5] = _split3(s_sq)

    for c in range(3):
        vh, vm, vl = _split3((-2.0 * w_pts[:, c]).astype(np.float32))
        ph, pm, pl = _split3(s_pts[:, c])
        r = 6 + 6 * c
        Wm[r + 0], S[r + 0] = vh, ph
        Wm[r + 1], S[r + 1] = vh, pm
        Wm[r + 2], S[r + 2] = vm, ph
        Wm[r + 3], S[r + 3] = vh, pl
        Wm[r + 4], S[r + 4] = vl, ph
        Wm[r + 5], S[r + 5] = vm, pm
    return Wm, S


def _replicate4(Aa):
    out = np.zeros((128, Aa.shape[1]), ml_dtypes.bfloat16)
    for i in range(4):
        out[32 * i : 32 * i + KROWS] = Aa
    return out


def _grid_witness(q, x, cs=0.25):
    """u[i] >= nn-dist upper bound: min dist from q[i] to x's in q[i]'s 3x3x3
    grid cell neighborhood (inf if empty)."""
    lo = np.floor(np.minimum(q.min(0), x.min(0)) / cs).astype(np.int64) - 2
    qc = np.floor(q / cs).astype(np.int64) - lo
    xc = np.floor(x / cs).astype(np.int64) - lo
    dim = int(max(qc.max(), xc.max())) + 3
    xid = (xc[:, 0] * dim + xc[:, 1]) * dim + xc[:, 2]
    order = np.argsort(xid, kind="stable")
    xid_s, x_s = xid[order], x[order]
    u2 = np.full(len(q), np.inf)
    for dx in (-1, 0, 1):
        for dy in (-1, 0, 1):
            for dz in (-1, 0, 1):
                qid = ((qc[:, 0] + dx) * dim + qc[:, 1] + dy) * dim + qc[:, 2] + dz
                beg = np.searchsorted(xid_s, qid, "left")
                end = np.searchsorted(xid_s, qid, "right")
                cnt = end - beg
                cmax = int(cnt.max()) if len(cnt) else 0
                for c in range(1, cmax + 1):
                    sel = np.nonzero(cnt == c)[0]
                    if len(sel) == 0:
                        continue
                    idx = beg[sel][:, None] + np.arange(c)[None, :]
                    d2 = ((q[sel][:, None, :] - x_s[idx]) ** 2).sum(-1).min(1)
                    u2[sel] = np.minimum(u2[sel], d2)
    return np.sqrt(u2)


def _hard_points(kq, kx, u, windows):
    """hard[i]: window (in x-rank space) not certified to contain q_i's nn.
    windows: (lo_rank, hi_rank) per q index."""
    lo_r, hi_r = windows
    # conservative: compare against the window's own endpoint keys
    lo_k = np.where(lo_r > 0, kx[np.clip(lo_r, 0, len(kx) - 1)], -np.inf)
    hi_k = np.where(
        hi_r < len(kx), kx[np.clip(hi_r - 1, 0, len(kx) - 1)], np.inf
    )
    return (kq - u < lo_k) | (kq + u > hi_k)


def prepare_batch(pred_b, targ_b):
    """-> (in_map, meta) for one batch."""
    p0 = np.asarray(pred_b, np.float32)
    t0 = np.asarray(targ_b, np.float32)
    kp0 = np.linalg.norm(p0, axis=1)
    kt0 = np.linalg.norm(t0, axis=1)
    po = np.argsort(kp0, kind="stable")
    to = np.argsort(kt0, kind="stable")
    p, t = p0[po], t0[to]
    kp, kt = kp0[po], kt0[to]

    u_p = _grid_witness(p, t)
    u_t = _grid_witness(t, p)

    Aw = np.array(A_WIN)
    # pred certificates: tile windows
    tile_lo = np.repeat(Aw, 128)
    tile_hi = np.repeat(Aw + W, 128)
    hard_p = _hard_points(kp, kt, u_p, (tile_lo, tile_hi))

    # target certificates: covering pred rank span
    m_idx = np.arange(M)
    lo_i = np.searchsorted(Aw, m_idx - W, "right")  # first tile with A > m-W
    hi_i = np.searchsorted(Aw, m_idx, "right") - 1  # last tile with A <= m
    cov_lo = np.where(lo_i == 0, -np.inf, kp[np.clip(128 * lo_i, 0, N - 1)])
    cov_hi = np.where(
        hi_i == NT - 1, np.inf, kp[np.clip(128 * hi_i + 127, 0, N - 1)]
    )
    hard_t = (kt - u_t < cov_lo) | (kt + u_t > cov_hi)

    hp = np.nonzero(hard_p)[0]
    ht = np.nonzero(hard_t)[0]
    assert len(hp) <= HARD_CAP and len(ht) <= HARD_CAP, (len(hp), len(ht))
    hp_pad = np.concatenate([hp, np.zeros(HARD_CAP - len(hp), np.int64)])
    ht_pad = np.concatenate([ht, np.zeros(HARD_CAP - len(ht), np.int64)])

    p_sq = (p.astype(np.float64) ** 2).sum(axis=1).astype(np.float32)
    t_sq = (t.astype(np.float64) ** 2).sum(axis=1).astype(np.float32)
    WA, SA = _aug24(p, t, p_sq, t_sq)  # orientation A: weights = preds
    WB, SB = _aug24(t, p, t_sq, p_sq)  # orientation B: weights = targets

    in_map = {
        "wa": _replicate4(WA),
        "sa": _replicate4(SA),
        "wah": _replicate4(WA[:, hp_pad]),
        "wbh": _replicate4(WB[:, ht_pad]),
        "sb": _replicate4(SB),
        "ident": np.eye(128, dtype=np.float16),
    }
    meta = {"po": po, "to": to, "hp": hp, "ht": ht}
    return in_map, meta


def combine(mins, meta):
    """[128, 144] device result + meta -> per-batch chamfer scalar (fp64)."""
    rowmin = mins[:, 0:64].T.reshape(-1).copy()  # sorted-pred order
    colmin = mins[:, 64:128].T.reshape(-1).copy()  # sorted-target order
    hp, ht = meta["hp"], meta["ht"]
    if len(hp):
        hard = mins[: len(hp), 128:136].min(axis=1)
        rowmin[hp] = np.minimum(rowmin[hp], hard)
    if len(ht):
        hard = mins[: len(ht), 136:144].min(axis=1)
        colmin[ht] = np.minimum(colmin[ht], hard)
    return np.maximum(rowmin, 0).mean(dtype=np.float64) + np.maximum(
        colmin, 0
    ).mean(dtype=np.float64)


_NC_CACHE = {}


def _get_nc():
    if "nc" not in _NC_CACHE:
        _NC_CACHE["nc"] = build_nc()
    return _NC_CACHE["nc"]


def kernel(pred: np.ndarray, target: np.ndarray) -> np.ndarray:
    pred = np.asarray(pred, np.float32)
    target = np.asarray(target, np.float32)
    assert pred.shape == (B, N, 3) and target.shape == (B, M, 3), (
        pred.shape,
        target.shape,
    )

    nc = _get_nc()
    prepped = [prepare_batch(pred[b], target[b]) for b in range(B)]
    in_maps = [pm[0] for pm in prepped]
    results = run_bass_kernel_spmd(nc, in_maps, list(range(B))).results

    cd = sum(combine(results[b]["mins"], prepped[b][1]) for b in range(B)) / B
    return np.array(cd, dtype=np.float32)
